# revision 6
# baseline (speedup 1.0000x reference)
"""Multi-Latent Attention TRN2 kernel.

Sharding: tensor-parallel over heads. 16 heads / 8 cores = 2 heads per core.
Each core computes its 2 heads' projections + attention and a partial of the
final output projection (contracting only its heads' feature block); the host
sums the 8 partials and adds the output bias.

On-device dataflow is feature-major (transposed): the host feeds X^T for
queries/keys/values so every matmul contracts along SBUF partitions.

  q^T   = Wq_c^T  X_q^T            [256, T]   (fp32r)
  latk^T= Wlk_c^T X_k^T            [128, T]   (fp32r)
  latv^T= Wlv_c^T X_v^T            [128, T]   (fp32r)
  k^T   = blockdiag(Wkr)^T latk^T  [256, T]   (fp32r, per head)
  v     = latv blockdiag(Wvr)      [T, 256]   (fp32r, token-major)
  S     = q k^T / sqrt(dk)  (bf16 matmul, fp32 psum)
  P~    = exp(S)  (no max subtraction: scores are O(1) by construction)
  U^T   = v^T P~^T (bf16; P~^T via PE transpose)
  attnout^T = U^T * recip(rowsum) + bvr
  out_partial = attnout @ Wo_rows  (bf16)

HW: all matmuls keep moving free dim >= 256 where fp32r is used (full rate).
"""

import math

import numpy as np

import concourse.mybir as mybir
from concourse import bacc
from concourse.bass import ds, ts
from concourse.masks import make_identity
from concourse.tile import TileContext

# Problem constants (hardcoded per contract).
B, S, D = 2, 2048, 2048
H, DK, DV, L = 16, 128, 128, 64
N_CORES = 8
HPC = H // N_CORES        # heads per core = 2
T = B * S                 # 4096 tokens
SB = S                    # tokens per batch
FPC = HPC * DK            # feature cols per core = 256
LPC = HPC * L             # latent cols per core = 128
CHUNK = 256               # token chunk for projection streaming
NCH = SB // CHUNK         # chunks per batch = 8
KO = D // 128             # contraction k-tiles for D = 16
QT = SB // 128            # 128-row query tiles per batch = 16

F32 = mybir.dt.float32
F32R = mybir.dt.float32r
BF16 = mybir.dt.bfloat16

INV_SQRT_DK = 1.0 / math.sqrt(DK)


def build_kernel():
    nc = bacc.Bacc(trn_type="TRN2", debug=False)

    # ---- DRAM I/O ----
    qT = nc.dram_tensor("qT", [D, T], F32R, kind="ExternalInput")
    kT = nc.dram_tensor("kT", [D, T], F32R, kind="ExternalInput")
    vT = nc.dram_tensor("vT", [D, T], F32R, kind="ExternalInput")
    wq = nc.dram_tensor("wq", [D, FPC], F32R, kind="ExternalInput")
    bq = nc.dram_tensor("bq", [FPC], F32, kind="ExternalInput")
    wlk = nc.dram_tensor("wlk", [D, LPC], F32R, kind="ExternalInput")
    blk = nc.dram_tensor("blk", [LPC], F32, kind="ExternalInput")
    wlv = nc.dram_tensor("wlv", [D, LPC], F32R, kind="ExternalInput")
    blv = nc.dram_tensor("blv", [LPC], F32, kind="ExternalInput")
    wkr2 = nc.dram_tensor("wkr2", [LPC, FPC], F32R, kind="ExternalInput")
    bkr = nc.dram_tensor("bkr", [DK], F32, kind="ExternalInput")
    wvr2 = nc.dram_tensor("wvr2", [LPC, FPC], F32R, kind="ExternalInput")
    bvr = nc.dram_tensor("bvr", [DV], F32, kind="ExternalInput")
    wo = nc.dram_tensor("wo", [FPC, D], F32, kind="ExternalInput")
    outp = nc.dram_tensor("outp", [T, D], F32, kind="ExternalOutput")

    from contextlib import ExitStack

    with TileContext(nc) as tc, ExitStack() as ctx:
        ec = ctx.enter_context
        consts = ec(tc.tile_pool(name="consts", bufs=1))
        persist = ec(tc.tile_pool(name="persist", bufs=1))
        xpool = ec(tc.tile_pool(name="xpool", bufs=2))
        latpool = ec(tc.tile_pool(name="latpool", bufs=3))
        ppool = ec(tc.tile_pool(name="ppool", bufs=2))
        ptpool = ec(tc.tile_pool(name="ptpool", bufs=2))
        rpool = ec(tc.tile_pool(name="rpool", bufs=2))
        statpool = ec(tc.tile_pool(name="statpool", bufs=4))
        opool = ec(tc.tile_pool(name="opool", bufs=2))
        ps512 = ec(tc.tile_pool(name="ps512", bufs=3, space="PSUM"))
        pst = ec(tc.tile_pool(name="pst", bufs=2, space="PSUM"))
        psu = ec(tc.tile_pool(name="psu", bufs=2, space="PSUM"))
        psr = ec(tc.tile_pool(name="psr", bufs=1, space="PSUM"))

        if True:
            # ---- constants / weights ----
            ident_bf = consts.tile([128, 128], BF16, tag="ident_bf")
            make_identity(nc, ident_bf)
            ident_f32 = consts.tile([128, 128], F32, tag="ident_f32")
            make_identity(nc, ident_f32)
            # causal mask for the diagonal 128x128 block: 1 where k <= q
            mask_sb = consts.tile([128, 128], BF16, tag="mask")
            nc.gpsimd.memset(mask_sb, 1.0)
            nc.gpsimd.affine_select(
                out=mask_sb, in_=mask_sb,
                compare_op=mybir.AluOpType.is_ge,
                fill=0.0, base=0, pattern=[[-1, 128]], channel_multiplier=1,
            )

            wq_sb = consts.tile([128, KO, FPC], F32R, tag="wq")
            nc.sync.dma_start(wq_sb, wq.rearrange("(ko p) m -> p ko m", p=128))
            wlk_sb = consts.tile([128, KO, LPC], F32R, tag="wlk")
            nc.sync.dma_start(wlk_sb, wlk.rearrange("(ko p) m -> p ko m", p=128))
            wlv_sb = consts.tile([128, KO, LPC], F32R, tag="wlv")
            nc.sync.dma_start(wlv_sb, wlv.rearrange("(ko p) m -> p ko m", p=128))
            wkr2_sb = consts.tile([128, FPC], F32R, tag="wkr2")
            nc.sync.dma_start(wkr2_sb, wkr2[:, :])
            wvr2_sb = consts.tile([128, FPC], F32R, tag="wvr2")
            nc.sync.dma_start(wvr2_sb, wvr2[:, :])
            wo_sb = consts.tile([128, HPC, D], BF16, tag="wo")
            nc.gpsimd.dma_start(wo_sb, wo.rearrange("(kk p) d -> p kk d", p=128))

            bq_sb = consts.tile([128, HPC], F32, tag="bq")
            nc.sync.dma_start(bq_sb, bq.rearrange("(m p) -> p m", p=128))
            blk_sb = consts.tile([128, 1], F32, tag="blk")
            nc.sync.dma_start(blk_sb, blk[:, None])
            blv_sb = consts.tile([128, 1], F32, tag="blv")
            nc.sync.dma_start(blv_sb, blv[:, None])
            bkr_sb = consts.tile([128, 1], F32, tag="bkr")
            nc.sync.dma_start(bkr_sb, bkr[:, None])
            bvr_sb = consts.tile([128, 1], F32, tag="bvr")
            nc.sync.dma_start(bvr_sb, bvr[:, None])

            # attnout^T (both batches), feature-major, lhsT of final matmul
            asb = persist.tile([128, HPC, T], BF16, tag="asb")

            qT_r = qT.rearrange("(ko p) t -> p ko t", p=128)
            kT_r = kT.rearrange("(ko p) t -> p ko t", p=128)
            vT_r = vT.rearrange("(ko p) t -> p ko t", p=128)

            for b in range(B):
                qsb = persist.tile([128, HPC, SB], BF16, tag=f"qsb{b}")
                ksb = persist.tile([128, HPC, SB], BF16, tag=f"ksb{b}")
                vsb = persist.tile([128, QT, FPC], BF16, tag=f"vsb{b}")

                # ---- projections, streamed over 256-token chunks ----
                for c in range(NCH):
                    t0 = b * SB + c * CHUNK  # global token start
                    csl = ds(c * CHUNK, CHUNK)

                    # q^T chunk
                    xq = xpool.tile([128, KO, CHUNK], F32R, tag="x")
                    nc.sync.dma_start(xq, qT_r[:, :, ds(t0, CHUNK)])
                    for m in range(HPC):
                        ps = ps512.tile([128, 512], F32, tag="s")
                        for ko in range(KO):
                            nc.tensor.matmul(
                                ps[:, :CHUNK],
                                wq_sb[:, ko, ts(m, 128)],
                                xq[:, ko, :],
                                start=(ko == 0), stop=(ko == KO - 1),
                            )
                        nc.scalar.activation(
                            qsb[:, m, csl], ps[:, :CHUNK],
                            mybir.ActivationFunctionType.Identity,
                            bias=bq_sb[:, m : m + 1],
                        )

                    # latk chunk -> k^T chunk (per head)
                    xk = xpool.tile([128, KO, CHUNK], F32R, tag="x")
                    nc.sync.dma_start(xk, kT_r[:, :, ds(t0, CHUNK)])
                    lk = latpool.tile([128, CHUNK], F32R, tag="lat")
                    ps = ps512.tile([128, 512], F32, tag="s")
                    for ko in range(KO):
                        nc.tensor.matmul(
                            ps[:, :CHUNK],
                            wlk_sb[:, ko, :],
                            xk[:, ko, :],
                            start=(ko == 0), stop=(ko == KO - 1),
                        )
                    nc.scalar.activation(
                        lk, ps[:, :CHUNK],
                        mybir.ActivationFunctionType.Identity,
                        bias=blk_sb[:, 0:1],
                    )
                    for h in range(HPC):
                        psk = ps512.tile([128, 512], F32, tag="s")
                        nc.tensor.matmul(
                            psk[:, :CHUNK],
                            wkr2_sb[:, ts(h, 128)],
                            lk,
                            start=True, stop=True,
                        )
                        nc.scalar.activation(
                            ksb[:, h, csl], psk[:, :CHUNK],
                            mybir.ActivationFunctionType.Identity,
                            bias=bkr_sb[:, 0:1],
                        )

                    # latv chunk -> v (token-major) chunk
                    xv = xpool.tile([128, KO, CHUNK], F32R, tag="x")
                    nc.sync.dma_start(xv, vT_r[:, :, ds(t0, CHUNK)])
                    lv = latpool.tile([128, CHUNK], F32R, tag="lat")
                    ps = ps512.tile([128, 512], F32, tag="s")
                    for ko in range(KO):
                        nc.tensor.matmul(
                            ps[:, :CHUNK],
                            wlv_sb[:, ko, :],
                            xv[:, ko, :],
                            start=(ko == 0), stop=(ko == KO - 1),
                        )
                    nc.scalar.activation(
                        lv, ps[:, :CHUNK],
                        mybir.ActivationFunctionType.Identity,
                        bias=blv_sb[:, 0:1],
                    )
                    for j2 in range(CHUNK // 128):
                        psv = ps512.tile([128, 512], F32, tag="s")
                        nc.tensor.matmul(
                            psv[:, :FPC],
                            lv[:, ts(j2, 128)],
                            wvr2_sb,
                            start=True, stop=True,
                        )
                        jt = (c * CHUNK) // 128 + j2
                        nc.any.tensor_copy(out=vsb[:, jt, :], in_=psv[:, :FPC])

                # ---- attention for this batch ----
                for h in range(HPC):
                    for i in range(QT):
                        kmax = 128 * (i + 1)
                        nfull = kmax // 512
                        rem = kmax - nfull * 512
                        q_ap = qsb[:, h, ts(i, 128)]

                        p_tile = ppool.tile([128, SB], BF16, tag="p")
                        for kc in range(nfull):
                            ps_s = ps512.tile([128, 512], F32, tag="s")
                            nc.tensor.matmul(
                                ps_s, q_ap, ksb[:, h, ts(kc, 512)],
                                start=True, stop=True,
                            )
                            nc.scalar.activation(
                                p_tile[:, ts(kc, 512)], ps_s,
                                mybir.ActivationFunctionType.Exp,
                                scale=INV_SQRT_DK,
                            )
                        if rem:
                            ps_s = ps512.tile([128, 512], F32, tag="s")
                            nc.tensor.matmul(
                                ps_s[:, :rem], q_ap,
                                ksb[:, h, ds(nfull * 512, rem)],
                                start=True, stop=True,
                            )
                            nc.scalar.activation(
                                p_tile[:, ds(nfull * 512, rem)], ps_s[:, :rem],
                                mybir.ActivationFunctionType.Exp,
                                scale=INV_SQRT_DK,
                            )
                        # causal mask on the diagonal block
                        nc.vector.tensor_tensor(
                            p_tile[:, ds(kmax - 128, 128)],
                            p_tile[:, ds(kmax - 128, 128)],
                            mask_sb, mybir.AluOpType.mult,
                        )
                        rs = statpool.tile([128, 1], F32, tag="rs")
                        nc.vector.tensor_reduce(
                            rs, p_tile[:, :kmax],
                            axis=mybir.AxisListType.X, op=mybir.AluOpType.add,
                        )
                        rcp = statpool.tile([128, 1], F32, tag="rcp")
                        nc.vector.reciprocal(rcp, rs)

                        # transpose P~ blocks, then U^T = v^T P~^T
                        pt_tile = ptpool.tile([128, QT, 128], BF16, tag="pt")
                        for j in range(i + 1):
                            ps_t = pst.tile([128, 128], BF16, tag="t")
                            nc.tensor.transpose(
                                ps_t, p_tile[:, ts(j, 128)], ident_bf
                            )
                            nc.any.tensor_copy(out=pt_tile[:, j, :], in_=ps_t)
                        ps_u = psu.tile([128, 128], F32, tag="u")
                        for j in range(i + 1):
                            nc.tensor.matmul(
                                ps_u, vsb[:, j, ts(h, 128)], pt_tile[:, j, :],
                                start=(j == 0), stop=(j == i),
                            )

                        # replicate recip along partitions: R[d, q] = rcp[q]
                        rt_sb = rpool.tile([128, 128], F32, tag="rt")
                        nc.vector.tensor_copy(
                            out=rt_sb, in_=rcp.to_broadcast((128, 128))
                        )
                        ps_r = psr.tile([128, 128], F32, tag="r")
                        nc.tensor.transpose(ps_r, rt_sb, ident_f32)
                        r_sb = rpool.tile([128, 128], F32, tag="rsb")
                        nc.any.tensor_copy(out=r_sb, in_=ps_r)

                        a_sl = asb[:, h, ds(b * SB + i * 128, 128)]
                        nc.vector.tensor_tensor(
                            a_sl, ps_u, r_sb, mybir.AluOpType.mult
                        )
                        nc.vector.tensor_scalar_add(a_sl, a_sl, bvr_sb[:, 0:1])

            # ---- final projection: out_partial = attnout @ wo ----
            for tt in range(T // 128):
                o_sb = opool.tile([128, D], F32, tag="o")
                for dc in range(D // 512):
                    ps_f = ps512.tile([128, 512], F32, tag="s")
                    for kk in range(HPC):
                        nc.tensor.matmul(
                            ps_f, asb[:, kk, ts(tt, 128)],
                            wo_sb[:, kk, ts(dc, 512)],
                            start=(kk == 0), stop=(kk == HPC - 1),
                        )
                    nc.any.tensor_copy(out=o_sb[:, ts(dc, 512)], in_=ps_f)
                nc.sync.dma_start(outp[ts(tt, 128), :], o_sb)

    nc.finalize()
    return nc


_NC_CACHE = None


def _get_nc():
    global _NC_CACHE
    if _NC_CACHE is None:
        _NC_CACHE = build_kernel()
    return _NC_CACHE


def _prep_in_maps(queries, keys, values, Wq, bq, Wlk, blk, Wlv, blv,
                  Wkr, bkr, Wvr, bvr, Wo, bo):
    f = np.float32
    qT = np.ascontiguousarray(queries.reshape(T, D).T, dtype=f)
    kT = np.ascontiguousarray(keys.reshape(T, D).T, dtype=f)
    vT = np.ascontiguousarray(values.reshape(T, D).T, dtype=f)

    wkr2 = np.zeros((LPC, FPC), f)
    wkr2[0:L, 0:DK] = Wkr
    wkr2[L : 2 * L, DK : 2 * DK] = Wkr
    wvr2 = np.zeros((LPC, FPC), f)
    wvr2[0:L, 0:DV] = Wvr
    wvr2[L : 2 * L, DV : 2 * DV] = Wvr

    in_maps = []
    for c in range(N_CORES):
        fsl = slice(c * FPC, (c + 1) * FPC)   # feature cols (q/k heads)
        lsl = slice(c * LPC, (c + 1) * LPC)   # latent cols
        in_maps.append({
            "qT": qT, "kT": kT, "vT": vT,
            "wq": np.ascontiguousarray(Wq[:, fsl], f),
            "bq": np.ascontiguousarray(bq[fsl], f),
            "wlk": np.ascontiguousarray(Wlk[:, lsl], f),
            "blk": np.ascontiguousarray(blk[lsl], f),
            "wlv": np.ascontiguousarray(Wlv[:, lsl], f),
            "blv": np.ascontiguousarray(blv[lsl], f),
            "wkr2": wkr2, "bkr": np.ascontiguousarray(bkr, f),
            "wvr2": wvr2, "bvr": np.ascontiguousarray(bvr, f),
            "wo": np.ascontiguousarray(Wo[fsl, :], f),
        })
    return in_maps


def kernel(**inputs):
    from concourse.bass_utils import run_bass_kernel_spmd

    nc = _get_nc()
    in_maps = _prep_in_maps(**inputs)
    res = run_bass_kernel_spmd(
        nc, in_maps, core_ids=list(range(N_CORES)), trace=False
    )
    acc = np.zeros((T, D), np.float64)
    for rmap in res.results:
        acc += rmap["outp"].astype(np.float64)
    acc += inputs["bo"].astype(np.float64)
    return acc.astype(np.float32).reshape(B, S, D)


if __name__ == "__main__":
    nc = build_kernel()
    print("built ok, instructions:", len(nc.inst_map))
